# revision 22
# baseline (speedup 1.0000x reference)
"""Trainium2 Bass kernel for nn_Arch7V4Layer (GNN message passing layer).

8-core SPMD decomposition:
  - flat nodes (320000) sharded 40000/core; canonical nodes (100000) sharded
    12500/core; roots/S (20000) sharded 2500/core.
  - intra edges sharded by dst range; global edges sharded by canonical dst
    range.  Segment sums are computed as one-hot matmuls over dst windows of
    128 nodes (host sorts edges by dst and emits a static window schedule,
    padded to the max across cores so all 8 cores run one identical graph).
  - gathers (h_flat[src], x_sum[src], T[node_ids], kk[sub_batch], roots) use
    gpsimd indirect DMA, 128 rows per call.
  - cross-core: AllGather of x_sum / kk_out / T tables, AllReduce of BN stats.

Self-contained: hardcodes shapes from the problem spec; builds the Bass graph
at call time from the actual index data.
"""

import os
import sys

sys.path.insert(0, "/opt/trn_rl_repo")

import numpy as np
import ml_dtypes

BF16NP = ml_dtypes.bfloat16

import bass_rust
from concourse import bass, mybir, tile
from concourse.bass_utils import run_bass_kernel_spmd

P = 128
H = 128
NC = 8
W_IN = 512   # intra scatter window (node columns per one-hot)
W_G = 512    # global scatter window
W_XS = 512   # x_sum scatter window
W_VV = 512   # vv scatter window
MAXW = 512
F32 = mybir.dt.float32
BF16 = mybir.dt.bfloat16
I32 = mybir.dt.int32


# ----------------------------------------------------------------- wait split
# This container's walrus rejects instructions carrying >1 sync wait; hoist
# extras onto standalone NoOps on the same engine (semantically identical).
_ws_ctr = [0]


def _split_multi_waits(nc):
    for bb in nc.m.functions[0].blocks:
        old = bb.instructions
        new = []
        for inst in old:
            si = inst.sync_info
            waits = list(si.on_wait) if si and si.on_wait else []
            if len(waits) > 1:
                hoistable = [w for w in waits if w.wait_reg is None]
                kept = [w for w in waits if w.wait_reg is not None]
                if not kept and hoistable:
                    kept = [hoistable.pop()]
                for w in hoistable:
                    _ws_ctr[0] += 1
                    nop = mybir.InstNoOp(name=f"WS-{_ws_ctr[0]}", ins=[], outs=[])
                    nop.engine = inst.engine
                    nop.sync_info = bass_rust.SyncInfo(
                        on_wait=[
                            mybir.SyncWait(
                                id=w.id,
                                wait_value=w.wait_value,
                                sync_type=w.sync_type,
                                wait_mode=w.wait_mode,
                            )
                        ],
                        on_update=[],
                    )
                    new.append(nop)
                inst.sync_info = bass_rust.SyncInfo(
                    on_wait=kept, on_update=list(si.on_update)
                )
            new.append(inst)
        bb.instructions = new


# ------------------------------------------------------------------ host prep
def _window_schedule(loc_dst_lists, nwin, win):
    """loc_dst_lists: per-core sorted local dst arrays. Returns per-window
    padded lengths L_w (cross-core max, >=1) and window boundaries."""
    counts = np.zeros((NC, nwin), np.int64)
    for c, dl in enumerate(loc_dst_lists):
        if len(dl):
            counts[c] = np.bincount(dl // win, minlength=nwin)
    L = counts.max(axis=0)
    # Sub-128 / partition-offset matmuls are broken in this toolchain, so every
    # scatter matmul is a full 128-row tile: pad windows to multiples of 128.
    L = ((np.maximum(L, 1) + P - 1) // P) * P
    bounds = np.concatenate([[0], np.cumsum(L)])
    return L, bounds


def _pack_rows(order_vals, T):
    """[T*128] -> [128, T] partition-major (entry (p,t) = element t*128+p)."""
    return np.ascontiguousarray(order_vals.reshape(T, P).T)


def _build_edge_arrays(src, dst_local, payload_rows, L, bounds, win, recipw=None):
    """Sort (src, dst_local, payload) into the padded window layout.

    Returns idx [128,T] int32, rel [128,T] f32, payload [128,T,H] f32 or None,
    w [128,T] f32 or None, T (number of 128-row tiles).
    """
    total = int(bounds[-1])
    T = (total + P - 1) // P
    tot_pad = T * P
    idx = np.zeros(tot_pad, np.int32)
    rel = np.full(tot_pad, -1.0, np.float32)
    pay = None
    if payload_rows is not None:
        pay = np.zeros((tot_pad, H), np.float32)
    wv = None
    if recipw is not None:
        wv = np.zeros(tot_pad, np.float32)
    if len(src):
        order = np.argsort(dst_local, kind="stable")
        s = src[order]
        d = dst_local[order]
        wi = d // win
        # position within the padded layout: bounds[wi] + rank within window
        cnts = np.bincount(wi, minlength=len(L))
        offs_in_win = np.arange(len(d)) - np.concatenate([[0], np.cumsum(cnts)])[wi]
        pos = bounds[wi] + offs_in_win
        idx[pos] = s
        rel[pos] = (d % win).astype(np.float32)
        if pay is not None:
            pay[pos] = payload_rows[order]
        if wv is not None:
            wv[pos] = recipw[order]
    out = {
        "idx": _pack_rows(idx, T),
        "rel": _pack_rows(rel, T).astype(np.float32),
        "T": T,
    }
    if pay is not None:
        out["pay"] = np.ascontiguousarray(
            pay.reshape(T, P, H).transpose(1, 0, 2)
        )
    if wv is not None:
        out["w"] = _pack_rows(wv, T).astype(np.float32)
    return out


def _matmul_ops(L, bounds, nwin):
    """Static list of scatter-matmul ops: (tile, a, b, window, start, stop)."""
    ops = []
    for w in range(nwin):
        lo, hi = int(bounds[w]), int(bounds[w] + L[w])
        t0, t1 = lo // P, (hi - 1) // P
        for t in range(t0, t1 + 1):
            a = max(lo, t * P) - t * P
            b = min(hi, (t + 1) * P) - t * P
            ops.append((t, a, b, w, t == t0, t == t1))
    return ops


def _prep(inputs, sizes):
    NF, NT, S, K, EI, EG = (
        sizes["NF"], sizes["NT"], sizes["S"], sizes["K"], sizes["EI"], sizes["EG"],
    )
    FLAT, CAN, SS = NF // NC, NT // NC, S // NC

    h_flat = np.asarray(inputs["h_flat"], np.float32)
    intra_ei = np.asarray(inputs["intra_ei"], np.int32)
    ea_flat = np.asarray(inputs["ea_flat"], np.float32)
    valid = np.asarray(inputs["valid"], bool)
    node_ids = np.asarray(inputs["node_ids"], np.int32)
    edge_index = np.asarray(inputs["edge_index"], np.int32)
    edge_attr = np.asarray(inputs["edge_attr"], np.float32)
    sub_batch = np.asarray(inputs["sub_batch"], np.int32)
    root_flat_idx = np.asarray(inputs["root_flat_idx"], np.int32)

    ids = np.maximum(node_ids, 0)
    vmask = node_ids >= 0
    cnt = np.bincount(ids[vmask], minlength=NT).astype(np.float32)
    recip = 1.0 / np.maximum(cnt, 1.0)
    root_ids_all = node_ids[root_flat_idx]
    rvalid = root_ids_all >= 0
    rids = np.maximum(root_ids_all, 0)
    rcnt = np.bincount(rids[rvalid], minlength=NT).astype(np.float32)
    rrecip = 1.0 / np.maximum(rcnt, 1.0)

    nwin_f = (FLAT + W_IN - 1) // W_IN
    nwin_c = (CAN + W_G - 1) // W_G
    nwin_xs = (CAN + W_XS - 1) // W_XS
    nwin_vv = (CAN + W_VV - 1) // W_VV

    # ---- per-core selections
    intra_sel = []
    glob_sel = []
    xs_sel = []
    vv_sel = []
    d_in = intra_ei[1]
    d_g = edge_index[1]
    for c in range(NC):
        lo, hi = c * FLAT, (c + 1) * FLAT
        e = np.where((d_in >= lo) & (d_in < hi))[0]
        intra_sel.append((intra_ei[0][e], d_in[e] - lo, e))
        lo, hi = c * CAN, (c + 1) * CAN
        e = np.where((d_g >= lo) & (d_g < hi))[0]
        glob_sel.append((edge_index[0][e], d_g[e] - lo, e))
        r = np.where(vmask & (ids >= lo) & (ids < hi))[0]
        xs_sel.append((r.astype(np.int32), ids[r] - lo))
        s = np.where(rvalid & (rids >= lo) & (rids < hi))[0]
        vv_sel.append((root_flat_idx[s].astype(np.int32), rids[s] - lo, s))

    L_in, B_in = _window_schedule([x[1] for x in intra_sel], nwin_f, W_IN)
    L_g, B_g = _window_schedule([x[1] for x in glob_sel], nwin_c, W_G)
    L_xs, B_xs = _window_schedule([x[1] for x in xs_sel], nwin_xs, W_XS)
    L_vv, B_vv = _window_schedule([x[1] for x in vv_sel], nwin_vv, W_VV)

    Tkk = (SS + P - 1) // P
    Tfin = (FLAT + P - 1) // P

    h16_full = h_flat.astype(BF16NP)
    in_maps = []
    for c in range(NC):
        m = {}
        m["hflat"] = h_flat
        m["h16"] = h16_full
        m["hflatT"] = np.ascontiguousarray(
            h_flat[c * FLAT : (c + 1) * FLAT].T
        ).astype(BF16NP)

        s_, dl, e = intra_sel[c]
        a = _build_edge_arrays(s_, dl, ea_flat[e], L_in, B_in, W_IN)
        m["idx_in"], m["rel_in"] = a["idx"], a["rel"]
        m["ea_in"] = a["pay"].astype(BF16NP)

        s_, dl, e = glob_sel[c]
        a = _build_edge_arrays(s_, dl, edge_attr[e], L_g, B_g, W_G)
        m["idx_g"], m["rel_g"] = a["idx"], a["rel"]
        m["ea_g"] = a["pay"].astype(BF16NP)

        r, dl = xs_sel[c]
        a = _build_edge_arrays(r, dl, None, L_xs, B_xs, W_XS, recipw=recip[ids[r]])
        m["idx_xs"], m["rel_xs"], m["w_xs"] = a["idx"], a["rel"], a["w"]

        r, dl, s = vv_sel[c]
        a = _build_edge_arrays(r, dl, None, L_vv, B_vv, W_VV, recipw=rrecip[rids[s]])
        m["idx_vv"], m["rel_vv"], m["w_vv"] = a["idx"], a["rel"], a["w"]

        kk_idx = np.zeros(Tkk * P, np.int32)
        kk_idx[:SS] = root_flat_idx[c * SS : (c + 1) * SS]
        m["idx_kk"] = _pack_rows(kk_idx, Tkk)

        fin_T = np.zeros(Tfin * P, np.int32)
        fin_K = np.zeros(Tfin * P, np.int32)
        fin_V = np.zeros(Tfin * P, np.float32)
        sl = slice(c * FLAT, (c + 1) * FLAT)
        fin_T[:FLAT] = ids[sl]
        fin_K[:FLAT] = sub_batch[sl]
        fin_V[:FLAT] = valid[sl].astype(np.float32)
        m["idx_T"] = _pack_rows(fin_T, Tfin)
        m["idx_K"] = _pack_rows(fin_K, Tfin)
        m["val_f"] = _pack_rows(fin_V, Tfin)

        m["ident"] = np.eye(P, dtype=np.float32)
        m["iota8"] = np.tile(np.arange(MAXW, dtype=np.float32), (P, 8))
        mats = np.stack(
            [
                np.asarray(inputs["local_w1"], np.float32),
                np.asarray(inputs["local_w2"], np.float32),
                np.asarray(inputs["global_w1"], np.float32),
                np.asarray(inputs["global_w2"], np.float32),
                np.asarray(inputs["skip_w"], np.float32),
                np.asarray(inputs["vv_w"], np.float32),
                np.asarray(inputs["kk_w"], np.float32),
            ],
            axis=1,
        )  # [H, 7, H]
        m["mats"] = np.ascontiguousarray(mats.reshape(H, 7 * H))
        cb = (
            np.asarray(inputs["skip_b"], np.float32)
            + np.asarray(inputs["vv_b"], np.float32)
            + np.asarray(inputs["kk_b"], np.float32)
        )
        vecs = np.stack(
            [
                np.asarray(inputs["local_b1"], np.float32),
                np.asarray(inputs["global_b1"], np.float32),
                np.asarray(inputs["local_gamma"], np.float32),
                np.asarray(inputs["local_beta"], np.float32),
                np.asarray(inputs["global_gamma"], np.float32),
                np.asarray(inputs["global_beta"], np.float32),
                cb,
                np.asarray(inputs["local_b2"], np.float32),
                np.asarray(inputs["global_b2"], np.float32),
            ],
            axis=1,
        )  # [H, 9]
        m["vecs"] = np.ascontiguousarray(vecs)
        in_maps.append(m)

    sched = {
        "sizes": sizes,
        "FLAT": FLAT, "CAN": CAN, "SS": SS,
        "nwin_f": nwin_f, "nwin_c": nwin_c,
        "ops_in": _matmul_ops(L_in, B_in, nwin_f),
        "ops_g": _matmul_ops(L_g, B_g, nwin_c),
        "ops_xs": _matmul_ops(L_xs, B_xs, nwin_xs),
        "ops_vv": _matmul_ops(L_vv, B_vv, nwin_vv),
        "T_in": in_maps[0]["idx_in"].shape[1],
        "T_g": in_maps[0]["idx_g"].shape[1],
        "T_xs": in_maps[0]["idx_xs"].shape[1],
        "T_vv": in_maps[0]["idx_vv"].shape[1],
        "T_kk": Tkk, "T_fin": Tfin,
    }
    return in_maps, sched


# ---------------------------------------------------------------- graph build
def _win_width(w, win, total):
    return min(win, total - w * win)


def _group_list(total, win=P):
    """Groups of up to 512//win windows (<=512 node columns each)."""
    nwin = (total + win - 1) // win
    per = max(1, 512 // win)
    groups = []
    w = 0
    while w < nwin:
        ws = list(range(w, min(w + per, nwin)))
        width = sum(_win_width(x, win, total) for x in ws)
        groups.append((ws, ws[0] * win, width))
        w += per
    return groups


def _build(sched, in_maps):
    sizes = sched["sizes"]
    NF, NT, S = sizes["NF"], sizes["NT"], sizes["S"]
    FLAT, CAN, SS = sched["FLAT"], sched["CAN"], sched["SS"]
    T_in, T_g, T_xs, T_vv = sched["T_in"], sched["T_g"], sched["T_xs"], sched["T_vv"]
    T_kk, T_fin = sched["T_kk"], sched["T_fin"]

    nc = bass.Bass()

    def din(name, arr):
        if arr.dtype == np.float32:
            dt = F32
        elif arr.dtype == BF16NP:
            dt = BF16
        else:
            dt = I32
        return nc.declare_dram_parameter(name, list(arr.shape), dt,
                                         isOutput=False)

    m0 = in_maps[0]
    hflat = din("hflat", m0["hflat"])
    h16 = din("h16", m0["h16"])
    hflatT = din("hflatT", m0["hflatT"])
    idx_in, rel_in, ea_in = din("idx_in", m0["idx_in"]), din("rel_in", m0["rel_in"]), din("ea_in", m0["ea_in"])
    idx_g, rel_g, ea_g = din("idx_g", m0["idx_g"]), din("rel_g", m0["rel_g"]), din("ea_g", m0["ea_g"])
    idx_xs, rel_xs, w_xs = din("idx_xs", m0["idx_xs"]), din("rel_xs", m0["rel_xs"]), din("w_xs", m0["w_xs"])
    idx_vv, rel_vv, w_vv = din("idx_vv", m0["idx_vv"]), din("rel_vv", m0["rel_vv"]), din("w_vv", m0["w_vv"])
    idx_kk = din("idx_kk", m0["idx_kk"])
    idx_T, idx_K, val_f = din("idx_T", m0["idx_T"]), din("idx_K", m0["idx_K"]), din("val_f", m0["val_f"])
    mats, vecs = din("mats", m0["mats"]), din("vecs", m0["vecs"])
    ident_d, iota_d = din("ident", m0["ident"]), din("iota8", m0["iota8"])
    out_ext = nc.declare_dram_parameter("out", [FLAT, H], F32, isOutput=True)

    # internal DRAM scratch
    t1T = nc.dram_tensor("t1T", [P, FLAT], BF16)
    t2T = nc.dram_tensor("t2T", [P, CAN], F32)
    xsT_loc = nc.dram_tensor("xsT_loc", [P, CAN], BF16)
    xs_bounce = nc.dram_tensor("xs_bounce", [CAN, H], BF16)
    xs_full = nc.dram_tensor("xs_full", [NT, H], BF16, addr_space="Shared")
    vvT = nc.dram_tensor("vvT", [P, CAN], F32)
    kk_bounce = nc.dram_tensor("kk_bounce", [SS, H], BF16)
    kk_full = nc.dram_tensor("kk_full", [S, H], BF16, addr_space="Shared")
    T_bounce = nc.dram_tensor("T_bounce", [CAN, H], BF16)
    T_full = nc.dram_tensor("T_full", [NT, H], BF16, addr_space="Shared")
    kg_stage = nc.dram_tensor("kg_stage", [FLAT, H], BF16)
    st_in = nc.dram_tensor("st_in", [P, 4], F32)
    st_out = nc.dram_tensor("st_out", [P, 4], F32, addr_space="Shared")

    RG = [list(range(NC))]
    NB = 8  # tiles per gather batch
    PH = os.environ.get("K_PHASES", "ABCDEF")

    with tile.TileContext(nc) as tc:
        with (
            tc.tile_pool(name="const", bufs=1) as cp,
            tc.tile_pool(name="stats", bufs=1) as statp,
        ):
            ident_t = cp.tile([P, P], F32)
            nc.sync.dma_start(out=ident_t[:], in_=ident_d[:])
            identb = cp.tile([P, P], BF16)
            nc.vector.tensor_copy(out=identb[:], in_=ident_t[:])
            mats_b = cp.tile([P, 7 * H], BF16)
            iota8 = cp.tile([P, NB * MAXW], F32)
            nc.sync.dma_start(out=iota8[:], in_=iota_d[:])
            mats_t = cp.tile([P, 7 * H], F32)
            nc.sync.dma_start(out=mats_t[:], in_=mats[:])
            nc.vector.tensor_copy(out=mats_b[:], in_=mats_t[:])
            vecs_t = cp.tile([P, 9], F32)
            nc.sync.dma_start(out=vecs_t[:], in_=vecs[:])
            W1L, W2L = mats_b[:, 0*H:1*H], mats_b[:, 1*H:2*H]
            W1G, W2G = mats_b[:, 2*H:3*H], mats_b[:, 3*H:4*H]
            WSK, WVV, WKK = mats_b[:, 4*H:5*H], mats_b[:, 5*H:6*H], mats_b[:, 6*H:7*H]
            B1L, B1G = vecs_t[:, 0:1], vecs_t[:, 1:2]
            GAL, BEL = vecs_t[:, 2:3], vecs_t[:, 3:4]
            GAG, BEG = vecs_t[:, 4:5], vecs_t[:, 5:6]
            CB = vecs_t[:, 6:7]

            # strips for BN statistics (sum / sum of squares per group)
            ngrp_f = len(_group_list(FLAT))
            ngrp_c = len(_group_list(CAN))
            s1_strip = statp.tile([P, ngrp_f], F32)
            q1_strip = statp.tile([P, ngrp_f], F32)
            s2_strip = statp.tile([P, ngrp_c], F32)
            q2_strip = statp.tile([P, ngrp_c], F32)

            # ============================================================
            # generic scatter-conv phase
            # ============================================================
            def scatter_phase(
                prefix, Ttiles, idx_d, rel_d, table, ops, total_cols,
                ea_d=None, w_d=None, win=P, psum_bufs=4, tdt=F32,
            ):
                """Gathers rows, builds messages, one-hot scatter-matmuls into
                PSUM groups; calls group_done(g, psum_ap, width, base) when a
                4-window group is complete."""
                with (
                    tc.tile_pool(name=prefix + "g", bufs=12) as gp,
                    tc.tile_pool(name=prefix + "m", bufs=8) as mp,
                    tc.tile_pool(name=prefix + "o", bufs=8) as op_,
                    tc.tile_pool(name=prefix + "i", bufs=1) as ip,
                    tc.tile_pool(name=prefix + "ps", bufs=psum_bufs, space="PSUM") as pp,
                ):
                    idx_t = ip.tile([P, Ttiles], I32)
                    nc.sync.dma_start(out=idx_t[:], in_=idx_d[:])
                    rel_t = ip.tile([P, Ttiles], F32)
                    nc.sync.dma_start(out=rel_t[:], in_=rel_d[:])
                    if w_d is not None:
                        w_t = ip.tile([P, Ttiles], F32)
                        nc.sync.dma_start(out=w_t[:], in_=w_d[:])

                    nbatch = (Ttiles + NB - 1) // NB
                    msgs_tiles = [None] * Ttiles
                    state = {"next": 0}

                    def issue_batch(b):
                        t0 = b * NB
                        nt = min(NB, Ttiles - t0)
                        g = gp.tile([P, NB, H], BF16, tag="g")
                        for j in range(nt):
                            nc.gpsimd.indirect_dma_start(
                                out=g[:, j, :], out_offset=None, in_=table[:],
                                in_offset=bass.IndirectOffsetOnAxis(
                                    ap=idx_t[:, t0 + j : t0 + j + 1], axis=0
                                ),
                            )
                        if ea_d is not None:
                            ea_t = mp.tile([P, NB, H], BF16, tag="ea")
                            nc.sync.dma_start(
                                out=ea_t[:, :nt, :], in_=ea_d[:, t0 : t0 + nt, :]
                            )
                            ms = mp.tile([P, NB, H], BF16, tag="ms")
                            nc.vector.tensor_tensor(
                                out=ms[:, :nt, :], in0=g[:, :nt, :],
                                in1=ea_t[:, :nt, :], op=mybir.AluOpType.add,
                            )
                            nc.scalar.activation(
                                out=ms[:, :nt, :], in_=ms[:, :nt, :],
                                func=mybir.ActivationFunctionType.Relu,
                            )
                        elif w_d is not None:
                            ms = mp.tile([P, NB, H], BF16, tag="ms")
                            nc.vector.tensor_tensor(
                                out=ms[:, :nt, :], in0=g[:, :nt, :],
                                in1=w_t[:, t0 : t0 + nt, None].to_broadcast(
                                    [P, nt, H]
                                ),
                                op=mybir.AluOpType.mult,
                            )
                        else:
                            ms = mp.tile([P, NB, H], BF16, tag="ms")
                            nc.vector.tensor_copy(out=ms[:, :nt, :],
                                                  in_=g[:, :nt, :])
                        oh = op_.tile([P, NB, win], BF16, tag="oh")
                        nc.vector.tensor_tensor(
                            out=oh[:, :nt, :],
                            in0=rel_t[:, t0 : t0 + nt, None].to_broadcast(
                                [P, nt, win]
                            ),
                            in1=iota8[:].rearrange("p (n q) -> p n q", n=NB)[
                                :, :nt, :win
                            ],
                            op=mybir.AluOpType.is_equal,
                        )
                        for j in range(nt):
                            msgs_tiles[t0 + j] = (ms, j, oh)

                    def ensure_tiles(tmax):
                        while state["next"] * NB <= tmax:
                            issue_batch(state["next"])
                            state["next"] += 1

                    groups = _group_list(total_cols, win)
                    opi = 0
                    for gi, (ws, base, width) in enumerate(groups):
                        ps = pp.tile([P, 512], F32, tag="agg")
                        for w in ws:
                            wslot = (w - ws[0]) * win
                            wn = _win_width(w, win, total_cols)
                            while opi < len(ops) and ops[opi][3] == w:
                                (t, a, b, _w, st, sp) = ops[opi]
                                ensure_tiles(t)
                                ms, j, oh = msgs_tiles[t]
                                nc.tensor.matmul(
                                    out=ps[:, wslot : wslot + wn],
                                    lhsT=ms[a:b, j, :],
                                    rhs=oh[a:b, j, :wn],
                                    start=st, stop=sp,
                                )
                                opi += 1
                        yield gi, ps, base, width

            # ============================================================
            # Phase A: x_sum build
            # ============================================================
            if "A" in PH:
              with (
                tc.tile_pool(name="xa", bufs=3) as xa,
                tc.tile_pool(name="xps", bufs=2, space="PSUM") as xps,
              ):
                for gi, ps, base, width in scatter_phase(
                    "xs", T_xs, idx_xs, rel_xs, h16, sched["ops_xs"], CAN,
                    w_d=w_xs, win=W_XS,
                ):
                    xs_sbb = xa.tile([P, 512], BF16, tag="xs_sbb")
                    nc.vector.tensor_copy(out=xs_sbb[:, :width],
                                          in_=ps[:, :width])
                    nc.scalar.dma_start(
                        out=xsT_loc[:, base : base + width],
                        in_=xs_sbb[:, :width]
                    )
                    # transpose to node-major rows for the AllGather table
                    nsub = (width + P - 1) // P
                    tr = xa.tile([P, nsub, P], BF16, tag="xs_tr")
                    for s in range(nsub):
                        wn = min(P, width - s * P)
                        pst = xps.tile([P, P], F32, tag="xs_t")
                        nc.tensor.matmul(
                            out=pst[:wn, :], lhsT=xs_sbb[:, s * P : s * P + wn],
                            rhs=identb[:], start=True, stop=True,
                        )
                        nc.scalar.activation(
                            out=tr[:wn, s, :], in_=pst[:wn, :],
                            func=mybir.ActivationFunctionType.Copy,
                        )
                        nc.scalar.dma_start(
                            out=xs_bounce[base + s * P : base + s * P + wn, :],
                            in_=tr[:wn, s, :],
                        )
              nc.gpsimd.collective_compute(
                "AllGather", mybir.AluOpType.bypass, ins=[xs_bounce[:].opt()],
                outs=[xs_full[:].opt()], replica_groups=RG,
              )

            if "D" in PH:
              with (
                tc.tile_pool(name="dv", bufs=3) as dv,
                tc.tile_pool(name="dps", bufs=1, space="PSUM") as dps,
              ):
                for gi, ps, base, width in scatter_phase(
                    "vv", T_vv, idx_vv, rel_vv, h16, sched["ops_vv"], CAN,
                    w_d=w_vv, win=W_VV, psum_bufs=2,
                ):
                    xv = dv.tile([P, 512], BF16, tag="xv")
                    nc.scalar.activation(
                        out=xv[:, :width], in_=ps[:, :width],
                        func=mybir.ActivationFunctionType.Copy,
                    )
                    psv = dps.tile([P, 512], F32, tag="psv")
                    nc.tensor.matmul(out=psv[:, :width], lhsT=WVV,
                                     rhs=xv[:, :width], start=True, stop=True)
                    vvc = dv.tile([P, 512], F32, tag="vvc")
                    nc.scalar.activation(
                        out=vvc[:, :width], in_=psv[:, :width],
                        func=mybir.ActivationFunctionType.Copy,
                    )
                    nc.scalar.dma_start(
                        out=vvT[:, base : base + width], in_=vvc[:, :width]
                    )

                # kk: gather root rows (node-major), transpose, matmul, store
                idxk_t = dv.tile([P, T_kk], I32, tag="idxk")
                nc.sync.dma_start(out=idxk_t[:], in_=idx_kk[:])
                for b in range((T_kk + NB - 1) // NB):
                    t0 = b * NB
                    nt = min(NB, T_kk - t0)
                    gk = dv.tile([P, NB, H], BF16, tag="gk")
                    for j in range(nt):
                        nc.gpsimd.indirect_dma_start(
                            out=gk[:, j, :], out_offset=None, in_=h16[:],
                            in_offset=bass.IndirectOffsetOnAxis(
                                ap=idxk_t[:, t0 + j : t0 + j + 1], axis=0
                            ),
                        )
                    for j in range(nt):
                        t = t0 + j
                        pstr = dps.tile([P, P], F32, tag="pstr")
                        nc.tensor.matmul(out=pstr[:], lhsT=gk[:, j, :],
                                         rhs=identb[:], start=True, stop=True)
                        gkT = dv.tile([P, P], BF16, tag="gkT")
                        nc.scalar.activation(
                            out=gkT[:], in_=pstr[:],
                            func=mybir.ActivationFunctionType.Copy,
                        )
                        psk = dps.tile([P, P], F32, tag="psk")
                        nc.tensor.matmul(out=psk[:], lhsT=gkT[:], rhs=WKK,
                                         start=True, stop=True)
                        kkr = dv.tile([P, P], BF16, tag="kkr")
                        nc.scalar.activation(
                            out=kkr[:], in_=psk[:],
                            func=mybir.ActivationFunctionType.Copy,
                        )
                        nrow = min(P, SS - t * P)
                        nc.scalar.dma_start(
                            out=kk_bounce[t * P : t * P + nrow, :],
                            in_=kkr[:nrow, :],
                        )
              nc.gpsimd.collective_compute(
                "AllGather", mybir.AluOpType.bypass, ins=[kk_bounce[:].opt()],
                outs=[kk_full[:].opt()], replica_groups=RG,
              )

            # ============================================================
            # Phase B: intra conv -> t1T + stats
            # ============================================================
            def mlp_phase(gen, W1, W2, B1, xT_src, tT_dst, strip_s, strip_q,
                          mlp_pool, mlp_psum, odt=F32):
                for gi, ps, base, width in gen:
                    xc = mlp_pool.tile([P, 512], BF16, tag="xc")
                    nc.sync.dma_start(
                        out=xc[:, :width], in_=xT_src[:, base : base + width]
                    )
                    hT = mlp_pool.tile([P, 512], BF16, tag="hT")
                    nc.vector.tensor_tensor(
                        out=hT[:, :width], in0=ps[:, :width], in1=xc[:, :width],
                        op=mybir.AluOpType.add,
                    )
                    psy = mlp_psum.tile([P, 512], F32, tag="psy")
                    nc.tensor.matmul(out=psy[:, :width], lhsT=W1,
                                     rhs=hT[:, :width], start=True, stop=True)
                    y1 = mlp_pool.tile([P, 512], BF16, tag="y1")
                    nc.scalar.activation(
                        out=y1[:, :width], in_=psy[:, :width],
                        func=mybir.ActivationFunctionType.Relu, bias=B1,
                    )
                    pst = mlp_psum.tile([P, 512], F32, tag="pst")
                    nc.tensor.matmul(out=pst[:, :width], lhsT=W2,
                                     rhs=y1[:, :width], start=True, stop=True)
                    tt = mlp_pool.tile([P, 512], odt, tag="tt")
                    nc.scalar.activation(
                        out=tt[:, :width], in_=pst[:, :width],
                        func=mybir.ActivationFunctionType.Copy,
                        accum_out=strip_s[:, gi : gi + 1],
                    )
                    sq = mlp_pool.tile([P, 512], odt, tag="sq")
                    nc.scalar.activation(
                        out=sq[:, :width], in_=tt[:, :width],
                        func=mybir.ActivationFunctionType.Square,
                        accum_out=strip_q[:, gi : gi + 1],
                    )
                    nc.scalar.dma_start(
                        out=tT_dst[:, base : base + width], in_=tt[:, :width]
                    )

            if "B" in PH:
              with (
                tc.tile_pool(name="bm", bufs=3) as bm,
                tc.tile_pool(name="bps", bufs=2, space="PSUM") as bps,
              ):
                mlp_phase(
                    scatter_phase("in", T_in, idx_in, rel_in, h16,
                                  sched["ops_in"], FLAT, ea_d=ea_in, win=W_IN),
                    W1L, W2L, B1L, hflatT, t1T, s1_strip, q1_strip, bm, bps,
                    odt=BF16,
                )

            # ============================================================
            # Phase C: global conv -> t2T + stats
            # ============================================================
            if "C" in PH:
              with (
                tc.tile_pool(name="cm", bufs=3) as cm,
                tc.tile_pool(name="cps", bufs=2, space="PSUM") as cps,
              ):
                mlp_phase(
                    scatter_phase("gl", T_g, idx_g, rel_g, xs_full,
                                  sched["ops_g"], CAN, ea_d=ea_g, win=W_G,
                                  tdt=BF16),
                    W1G, W2G, B1G, xsT_loc, t2T, s2_strip, q2_strip, cm, cps,
                )

            # ============================================================
            # Phase D: vv scatter-mean + matmul; kk rows
            # ============================================================
            # ============================================================
            # Phase E: BN stats AllReduce, T build, AG T
            # ============================================================
            if "E" in PH:
              with (
                tc.tile_pool(name="ev", bufs=2) as ev,
                tc.tile_pool(name="ekg", bufs=4) as ekg,
                tc.tile_pool(name="efi", bufs=1) as efi,
                tc.tile_pool(name="eps", bufs=2, space="PSUM") as eps,
              ):
                idxT_t = efi.tile([P, T_fin], I32)
                nc.sync.dma_start(out=idxT_t[:], in_=idx_T[:])
                idxK_t = efi.tile([P, T_fin], I32)
                nc.sync.dma_start(out=idxK_t[:], in_=idx_K[:])
                valf_t = efi.tile([P, T_fin], F32)
                nc.sync.dma_start(out=valf_t[:], in_=val_f[:])

                def kg_stage_tiles(tlo, thi):
                    """Gather kk rows (bf16) into DRAM staging, batched."""
                    for b0 in range(tlo, thi, NB):
                        nt = min(NB, thi - b0)
                        kg = ekg.tile([P, NB, H], BF16, tag="kg")
                        for j in range(nt):
                            nc.gpsimd.indirect_dma_start(
                                out=kg[:, j, :], out_offset=None,
                                in_=kk_full[:],
                                in_offset=bass.IndirectOffsetOnAxis(
                                    ap=idxK_t[:, b0 + j : b0 + j + 1], axis=0),
                            )
                        r0 = b0 * P
                        rows = min(nt * P, FLAT - r0)
                        if rows == nt * P:
                            nc.scalar.dma_start(
                                out=kg_stage[r0 : r0 + rows, :].rearrange(
                                    "(j p) h -> p j h", p=P),
                                in_=kg[:, :nt, :],
                            )
                        else:
                            for j in range(nt):
                                nr = min(P, FLAT - r0 - j * P)
                                if nr <= 0:
                                    break
                                nc.scalar.dma_start(
                                    out=kg_stage[
                                        r0 + j * P : r0 + j * P + nr, :
                                    ],
                                    in_=kg[:nr, j, :],
                                )

                st = ev.tile([P, 4], F32, tag="st")
                nc.vector.tensor_reduce(out=st[:, 0:1], in_=s1_strip[:],
                                        axis=mybir.AxisListType.X,
                                        op=mybir.AluOpType.add)
                nc.vector.tensor_reduce(out=st[:, 1:2], in_=q1_strip[:],
                                        axis=mybir.AxisListType.X,
                                        op=mybir.AluOpType.add)
                nc.vector.tensor_reduce(out=st[:, 2:3], in_=s2_strip[:],
                                        axis=mybir.AxisListType.X,
                                        op=mybir.AluOpType.add)
                nc.vector.tensor_reduce(out=st[:, 3:4], in_=q2_strip[:],
                                        axis=mybir.AxisListType.X,
                                        op=mybir.AluOpType.add)
                nc.sync.dma_start(out=st_in[:], in_=st[:])
                nc.gpsimd.collective_compute(
                    "AllReduce", mybir.AluOpType.add, ins=[st_in[:].opt()],
                    outs=[st_out[:].opt()], replica_groups=RG,
                )
                # keep Pool busy during the AR + BN-math + T-build chain
                KG1 = min(T_fin, (T_fin // 2 + NB - 1) // NB * NB)
                kg_stage_tiles(0, KG1)
                sg = ev.tile([P, 4], F32, tag="sg")
                nc.sync.dma_start(out=sg[:], in_=st_out[:])
                # stats -> scale/shift vectors  [P,1] each
                wk = ev.tile([P, 10], F32, tag="wk")
                EPS = 1e-5
                # mu1 = sg0/NF ; var1 = sg1/NF - mu1^2
                nc.vector.tensor_scalar(out=wk[:, 0:1], in0=sg[:, 0:1],
                                        scalar1=1.0 / NF, scalar2=None,
                                        op0=mybir.AluOpType.mult)
                nc.vector.tensor_scalar(out=wk[:, 1:2], in0=sg[:, 1:2],
                                        scalar1=1.0 / NF, scalar2=None,
                                        op0=mybir.AluOpType.mult)
                nc.vector.tensor_tensor(out=wk[:, 2:3], in0=wk[:, 0:1],
                                        in1=wk[:, 0:1], op=mybir.AluOpType.mult)
                nc.vector.tensor_tensor(out=wk[:, 2:3], in0=wk[:, 1:2],
                                        in1=wk[:, 2:3],
                                        op=mybir.AluOpType.subtract)
                # rs1 = 1/sqrt(var1+eps)
                nc.vector.tensor_scalar(out=wk[:, 2:3], in0=wk[:, 2:3],
                                        scalar1=EPS, scalar2=None,
                                        op0=mybir.AluOpType.add)
                nc.scalar.activation(out=wk[:, 3:4], in_=wk[:, 2:3],
                                     func=mybir.ActivationFunctionType.Sqrt)
                nc.vector.reciprocal(out=wk[:, 3:4], in_=wk[:, 3:4])
                # s1 = gamma1*rs1 ; c1 = beta1 - mu1*s1
                s1v = ev.tile([P, 1], F32, tag="s1v")
                nc.vector.tensor_tensor(out=s1v[:], in0=GAL, in1=wk[:, 3:4],
                                        op=mybir.AluOpType.mult)
                c1v = ev.tile([P, 1], F32, tag="c1v")
                nc.vector.tensor_tensor(out=c1v[:], in0=wk[:, 0:1], in1=s1v[:],
                                        op=mybir.AluOpType.mult)
                nc.vector.tensor_tensor(out=c1v[:], in0=BEL, in1=c1v[:],
                                        op=mybir.AluOpType.subtract)
                # same for stats2
                nc.vector.tensor_scalar(out=wk[:, 4:5], in0=sg[:, 2:3],
                                        scalar1=1.0 / NT, scalar2=None,
                                        op0=mybir.AluOpType.mult)
                nc.vector.tensor_scalar(out=wk[:, 5:6], in0=sg[:, 3:4],
                                        scalar1=1.0 / NT, scalar2=None,
                                        op0=mybir.AluOpType.mult)
                nc.vector.tensor_tensor(out=wk[:, 6:7], in0=wk[:, 4:5],
                                        in1=wk[:, 4:5], op=mybir.AluOpType.mult)
                nc.vector.tensor_tensor(out=wk[:, 6:7], in0=wk[:, 5:6],
                                        in1=wk[:, 6:7],
                                        op=mybir.AluOpType.subtract)
                nc.vector.tensor_scalar(out=wk[:, 6:7], in0=wk[:, 6:7],
                                        scalar1=EPS, scalar2=None,
                                        op0=mybir.AluOpType.add)
                nc.scalar.activation(out=wk[:, 7:8], in_=wk[:, 6:7],
                                     func=mybir.ActivationFunctionType.Sqrt)
                nc.vector.reciprocal(out=wk[:, 7:8], in_=wk[:, 7:8])
                s2v = ev.tile([P, 1], F32, tag="s2v")
                nc.vector.tensor_tensor(out=s2v[:], in0=GAG, in1=wk[:, 7:8],
                                        op=mybir.AluOpType.mult)
                c2v = ev.tile([P, 1], F32, tag="c2v")
                nc.vector.tensor_tensor(out=c2v[:], in0=wk[:, 4:5], in1=s2v[:],
                                        op=mybir.AluOpType.mult)
                nc.vector.tensor_tensor(out=c2v[:], in0=BEG, in1=c2v[:],
                                        op=mybir.AluOpType.subtract)
                # Tconst = c2 + c1 + (skip_b+vv_b+kk_b)
                tcv = ev.tile([P, 1], F32, tag="tcv")
                nc.vector.tensor_tensor(out=tcv[:], in0=c2v[:], in1=c1v[:],
                                        op=mybir.AluOpType.add)
                nc.vector.tensor_tensor(out=tcv[:], in0=tcv[:], in1=CB,
                                        op=mybir.AluOpType.add)

                # T = s2*t2 + Tconst + vv  (feature-major), transpose, store
                for (ws, base, width) in _group_list(CAN):
                    t2c = ev.tile([P, 512], F32, tag="t2c")
                    nc.sync.dma_start(out=t2c[:, :width],
                                      in_=t2T[:, base : base + width])
                    vvc2 = ev.tile([P, 512], F32, tag="vvc2")
                    nc.sync.dma_start(out=vvc2[:, :width],
                                      in_=vvT[:, base : base + width])
                    nc.vector.tensor_scalar(
                        out=t2c[:, :width], in0=t2c[:, :width],
                        scalar1=s2v[:], scalar2=tcv[:],
                        op0=mybir.AluOpType.mult, op1=mybir.AluOpType.add,
                    )
                    nc.vector.tensor_tensor(out=t2c[:, :width],
                                            in0=t2c[:, :width],
                                            in1=vvc2[:, :width],
                                            op=mybir.AluOpType.add)
                    t2b = ev.tile([P, 512], BF16, tag="t2b")
                    nc.vector.tensor_copy(out=t2b[:, :width], in_=t2c[:, :width])
                    nsub = (width + P - 1) // P
                    trT = ev.tile([P, 4, P], BF16, tag="trT")
                    for s in range(nsub):
                        wn = min(P, width - s * P)
                        pst2 = eps.tile([P, P], F32, tag="pst2")
                        nc.tensor.matmul(out=pst2[:wn, :],
                                         lhsT=t2b[:, s * P : s * P + wn],
                                         rhs=identb[:], start=True, stop=True)
                        nc.scalar.activation(
                            out=trT[:wn, s, :], in_=pst2[:wn, :],
                            func=mybir.ActivationFunctionType.Copy,
                        )
                        nc.scalar.dma_start(
                            out=T_bounce[base + s * P : base + s * P + wn, :],
                            in_=trT[:wn, s, :],
                        )
                nc.gpsimd.collective_compute(
                    "AllGather", mybir.AluOpType.bypass, ins=[T_bounce[:].opt()],
                    outs=[T_full[:].opt()], replica_groups=RG,
                )
                # rest of the kg staging runs while the T AllGather is in
                # flight
                kg_stage_tiles(KG1, T_fin)

                # ========================================================
                # Phase F: final fuse, NBF tiles per gather batch
                # ========================================================
                if "F" in PH:
                  NBF = 8
                  with (
                    tc.tile_pool(name="fv", bufs=3) as fv,
                    tc.tile_pool(name="fo", bufs=4) as fo,
                    tc.tile_pool(name="fps", bufs=4, space="PSUM") as fps,
                  ):
                    nsg = (T_fin + NBF - 1) // NBF

                    def sg_tiles(sg):
                        t0 = sg * NBF
                        return t0, min(NBF, T_fin - t0)

                    for sg in range(nsg):
                        t0, nt = sg_tiles(sg)
                        tg = fv.tile([P, NBF, H], BF16, tag="tg")
                        for j in range(nt):
                            nc.gpsimd.indirect_dma_start(
                                out=tg[:, j, :], out_offset=None, in_=T_full[:],
                                in_offset=bass.IndirectOffsetOnAxis(
                                    ap=idxT_t[:, t0 + j : t0 + j + 1], axis=0),
                            )
                        kg = fv.tile([P, NBF, H], BF16, tag="kg")
                        r0k = t0 * P
                        rowsk = min(nt * P, FLAT - r0k)
                        if rowsk == nt * P:
                            nc.sync.dma_start(
                                out=kg[:, :nt, :],
                                in_=kg_stage[r0k : r0k + rowsk, :].rearrange(
                                    "(j p) h -> p j h", p=P),
                            )
                        else:
                            for j in range(nt):
                                nr = min(P, FLAT - r0k - j * P)
                                if nr <= 0:
                                    break
                                nc.sync.dma_start(
                                    out=kg[:nr, j, :],
                                    in_=kg_stage[
                                        r0k + j * P : r0k + j * P + nr, :
                                    ],
                                )
                        ncol = min(NBF * P, FLAT - t0 * P)
                        xc2 = fv.tile([P, NBF * P], BF16, tag="xc2")
                        nc.sync.dma_start(
                            out=xc2[:, :ncol],
                            in_=hflatT[:, t0 * P : t0 * P + ncol])
                        t1c = fv.tile([P, NBF * P], BF16, tag="t1c")
                        nc.sync.dma_start(
                            out=t1c[:, :ncol],
                            in_=t1T[:, t0 * P : t0 * P + ncol])
                        y1c = fv.tile([P, NBF * P], BF16, tag="y1c")
                        nc.scalar.activation(
                            out=y1c[:, :ncol], in_=t1c[:, :ncol],
                            func=mybir.ActivationFunctionType.Copy,
                            scale=s1v[:],
                        )
                        for q in range(0, nt, 4):
                            qt = min(4, nt - q)
                            psf = fps.tile([P, 512], F32, tag="psf")
                            for j in range(q, q + qt):
                                c = (j - q) * P
                                nc.tensor.matmul(
                                    out=psf[:, c : c + P],
                                    lhsT=xc2[:, j * P : (j + 1) * P], rhs=WSK,
                                    start=True, stop=False)
                                nc.tensor.matmul(
                                    out=psf[:, c : c + P],
                                    lhsT=y1c[:, j * P : (j + 1) * P],
                                    rhs=identb[:], start=False, stop=True)
                            u = fo.tile([P, 4, H], F32, tag="u")
                            nc.vector.tensor_tensor(
                                out=u[:, :qt, :], in0=tg[:, q : q + qt, :],
                                in1=kg[:, q : q + qt, :],
                                op=mybir.AluOpType.add)
                            nc.vector.tensor_tensor(
                                out=u[:, :qt, :], in0=u[:, :qt, :],
                                in1=psf[:, : qt * P].rearrange(
                                    "p (j h) -> p j h", j=qt),
                                op=mybir.AluOpType.add)
                            nc.vector.tensor_tensor(
                                out=u[:, :qt, :], in0=u[:, :qt, :],
                                in1=valf_t[
                                    :, t0 + q : t0 + q + qt, None
                                ].to_broadcast([P, qt, H]),
                                op=mybir.AluOpType.mult)
                            ob = fo.tile([P, 4, H], F32, tag="ob")
                            nc.scalar.activation(
                                out=ob[:, :qt, :], in_=u[:, :qt, :],
                                func=mybir.ActivationFunctionType.Relu,
                            )
                            r0 = (t0 + q) * P
                            rows = min(qt * P, FLAT - r0)
                            if rows == qt * P:
                                nc.scalar.dma_start(
                                    out=out_ext[r0 : r0 + rows, :].rearrange(
                                        "(j p) h -> p j h", p=P),
                                    in_=ob[:, :qt, :],
                                )
                            else:
                                for j in range(qt):
                                    nr = min(P, FLAT - r0 - j * P)
                                    if nr <= 0:
                                        break
                                    nc.scalar.dma_start(
                                        out=out_ext[
                                            r0 + j * P : r0 + j * P + nr, :
                                        ],
                                        in_=ob[:nr, j, :],
                                    )

    _split_multi_waits(nc)
    return nc


# --------------------------------------------------------------------- driver
_DEFAULT_SIZES = dict(NF=320000, NT=100000, S=20000, K=16, EI=1280000, EG=800000)


LAST = {"exec_time_ns": None}


def kernel(sizes=None, **inputs):
    sizes = dict(_DEFAULT_SIZES if sizes is None else sizes)
    in_maps, sched = _prep(inputs, sizes)
    nc = _build(sched, in_maps)
    trace = bool(os.environ.get("K_TRACE"))
    kw = {}
    if trace and os.environ.get("K_TRACEDIR"):
        kw["tmpdir"] = os.environ["K_TRACEDIR"]
    res = run_bass_kernel_spmd(nc, in_maps, core_ids=list(range(NC)), trace=trace,
                               **kw)
    LAST["exec_time_ns"] = res.exec_time_ns
    LAST["res"] = res
    outs = [np.asarray(r["out"]) for r in res.results]
    return np.concatenate(outs, axis=0).astype(np.float32)



# revision 24
# speedup vs baseline: 1.0036x; 1.0036x over previous
"""Trainium2 Bass kernel for nn_Arch7V4Layer (GNN message passing layer).

8-core SPMD decomposition:
  - flat nodes (320000) sharded 40000/core; canonical nodes (100000) sharded
    12500/core; roots/S (20000) sharded 2500/core.
  - intra edges sharded by dst range; global edges sharded by canonical dst
    range.  Segment sums are computed as one-hot matmuls over dst windows of
    512 nodes (host sorts edges by dst and emits a static window schedule,
    padded to the max across cores so all 8 cores run one identical graph).
  - gathers (h[src], x_sum[src], T[node_ids], kk[sub_batch], roots) use
    gpsimd indirect DMA, 128 rows per call; the kernel is paced by the Pool
    engine's ~1.4-1.8 us per indirect call (~3.2k calls/core), so everything
    else (tensor, DVE, ACT, sequential DMA, collectives) hides under it.
  - bf16 everywhere off the critical numeric path: a host-precast bf16 copy
    of h_flat serves all h gathers; x_sum / T / kk cross-core tables, their
    AllGathers, edge payloads, hflatT and t1 are bf16 (f32 accumulation in
    PSUM and BN stats).
  - cross-core: AllGather of x_sum / kk_out / T tables (bf16), AllReduce of
    BN stats.  The kk[sub_batch] rows are staged to DRAM while the BN
    AllReduce + T build + T AllGather chain runs so Pool never idles; the
    final fuse then streams kk back sequentially.

Self-contained: hardcodes shapes from the problem spec; builds the Bass graph
at call time from the actual index data.
"""

import os
import sys

sys.path.insert(0, "/opt/trn_rl_repo")

import numpy as np
import ml_dtypes

BF16NP = ml_dtypes.bfloat16

import bass_rust
from concourse import bass, mybir, tile
from concourse.bass_utils import run_bass_kernel_spmd

P = 128
H = 128
NC = 8
W_IN = 512   # intra scatter window (node columns per one-hot)
W_G = 512    # global scatter window
W_XS = 512   # x_sum scatter window
W_VV = 512   # vv scatter window
MAXW = 512
F32 = mybir.dt.float32
BF16 = mybir.dt.bfloat16
I32 = mybir.dt.int32


# ----------------------------------------------------------------- wait split
# This container's walrus rejects instructions carrying >1 sync wait; hoist
# extras onto standalone NoOps on the same engine (semantically identical).
_ws_ctr = [0]


def _split_multi_waits(nc):
    for bb in nc.m.functions[0].blocks:
        old = bb.instructions
        new = []
        for inst in old:
            si = inst.sync_info
            waits = list(si.on_wait) if si and si.on_wait else []
            if len(waits) > 1:
                hoistable = [w for w in waits if w.wait_reg is None]
                kept = [w for w in waits if w.wait_reg is not None]
                if not kept and hoistable:
                    kept = [hoistable.pop()]
                for w in hoistable:
                    _ws_ctr[0] += 1
                    nop = mybir.InstNoOp(name=f"WS-{_ws_ctr[0]}", ins=[], outs=[])
                    nop.engine = inst.engine
                    nop.sync_info = bass_rust.SyncInfo(
                        on_wait=[
                            mybir.SyncWait(
                                id=w.id,
                                wait_value=w.wait_value,
                                sync_type=w.sync_type,
                                wait_mode=w.wait_mode,
                            )
                        ],
                        on_update=[],
                    )
                    new.append(nop)
                inst.sync_info = bass_rust.SyncInfo(
                    on_wait=kept, on_update=list(si.on_update)
                )
            new.append(inst)
        bb.instructions = new


# ------------------------------------------------------------------ host prep
def _window_schedule(loc_dst_lists, nwin, win):
    """loc_dst_lists: per-core sorted local dst arrays. Returns per-window
    padded lengths L_w (cross-core max, >=1) and window boundaries."""
    counts = np.zeros((NC, nwin), np.int64)
    for c, dl in enumerate(loc_dst_lists):
        if len(dl):
            counts[c] = np.bincount(dl // win, minlength=nwin)
    L = counts.max(axis=0)
    # Sub-128 / partition-offset matmuls are broken in this toolchain, so every
    # scatter matmul is a full 128-row tile: pad windows to multiples of 128.
    L = ((np.maximum(L, 1) + P - 1) // P) * P
    bounds = np.concatenate([[0], np.cumsum(L)])
    return L, bounds


def _pack_rows(order_vals, T):
    """[T*128] -> [128, T] partition-major (entry (p,t) = element t*128+p)."""
    return np.ascontiguousarray(order_vals.reshape(T, P).T)


def _build_edge_arrays(src, dst_local, payload_rows, L, bounds, win, recipw=None):
    """Sort (src, dst_local, payload) into the padded window layout.

    Returns idx [128,T] int32, rel [128,T] f32, payload [128,T,H] f32 or None,
    w [128,T] f32 or None, T (number of 128-row tiles).
    """
    total = int(bounds[-1])
    T = (total + P - 1) // P
    tot_pad = T * P
    idx = np.zeros(tot_pad, np.int32)
    rel = np.full(tot_pad, -1.0, np.float32)
    pay = None
    if payload_rows is not None:
        pay = np.zeros((tot_pad, H), np.float32)
    wv = None
    if recipw is not None:
        wv = np.zeros(tot_pad, np.float32)
    if len(src):
        order = np.argsort(dst_local, kind="stable")
        s = src[order]
        d = dst_local[order]
        wi = d // win
        # position within the padded layout: bounds[wi] + rank within window
        cnts = np.bincount(wi, minlength=len(L))
        offs_in_win = np.arange(len(d)) - np.concatenate([[0], np.cumsum(cnts)])[wi]
        pos = bounds[wi] + offs_in_win
        idx[pos] = s
        rel[pos] = (d % win).astype(np.float32)
        if pay is not None:
            pay[pos] = payload_rows[order]
        if wv is not None:
            wv[pos] = recipw[order]
    out = {
        "idx": _pack_rows(idx, T),
        "rel": _pack_rows(rel, T).astype(np.float32),
        "T": T,
    }
    if pay is not None:
        out["pay"] = np.ascontiguousarray(
            pay.reshape(T, P, H).transpose(1, 0, 2)
        )
    if wv is not None:
        out["w"] = _pack_rows(wv, T).astype(np.float32)
    return out


def _matmul_ops(L, bounds, nwin):
    """Static list of scatter-matmul ops: (tile, a, b, window, start, stop)."""
    ops = []
    for w in range(nwin):
        lo, hi = int(bounds[w]), int(bounds[w] + L[w])
        t0, t1 = lo // P, (hi - 1) // P
        for t in range(t0, t1 + 1):
            a = max(lo, t * P) - t * P
            b = min(hi, (t + 1) * P) - t * P
            ops.append((t, a, b, w, t == t0, t == t1))
    return ops


def _prep(inputs, sizes):
    NF, NT, S, K, EI, EG = (
        sizes["NF"], sizes["NT"], sizes["S"], sizes["K"], sizes["EI"], sizes["EG"],
    )
    FLAT, CAN, SS = NF // NC, NT // NC, S // NC

    h_flat = np.asarray(inputs["h_flat"], np.float32)
    intra_ei = np.asarray(inputs["intra_ei"], np.int32)
    ea_flat = np.asarray(inputs["ea_flat"], np.float32)
    valid = np.asarray(inputs["valid"], bool)
    node_ids = np.asarray(inputs["node_ids"], np.int32)
    edge_index = np.asarray(inputs["edge_index"], np.int32)
    edge_attr = np.asarray(inputs["edge_attr"], np.float32)
    sub_batch = np.asarray(inputs["sub_batch"], np.int32)
    root_flat_idx = np.asarray(inputs["root_flat_idx"], np.int32)

    ids = np.maximum(node_ids, 0)
    vmask = node_ids >= 0
    cnt = np.bincount(ids[vmask], minlength=NT).astype(np.float32)
    recip = 1.0 / np.maximum(cnt, 1.0)
    root_ids_all = node_ids[root_flat_idx]
    rvalid = root_ids_all >= 0
    rids = np.maximum(root_ids_all, 0)
    rcnt = np.bincount(rids[rvalid], minlength=NT).astype(np.float32)
    rrecip = 1.0 / np.maximum(rcnt, 1.0)

    nwin_f = (FLAT + W_IN - 1) // W_IN
    nwin_c = (CAN + W_G - 1) // W_G
    nwin_xs = (CAN + W_XS - 1) // W_XS
    nwin_vv = (CAN + W_VV - 1) // W_VV

    # ---- per-core selections
    intra_sel = []
    glob_sel = []
    xs_sel = []
    vv_sel = []
    d_in = intra_ei[1]
    d_g = edge_index[1]
    for c in range(NC):
        lo, hi = c * FLAT, (c + 1) * FLAT
        e = np.where((d_in >= lo) & (d_in < hi))[0]
        intra_sel.append((intra_ei[0][e], d_in[e] - lo, e))
        lo, hi = c * CAN, (c + 1) * CAN
        e = np.where((d_g >= lo) & (d_g < hi))[0]
        glob_sel.append((edge_index[0][e], d_g[e] - lo, e))
        r = np.where(vmask & (ids >= lo) & (ids < hi))[0]
        xs_sel.append((r.astype(np.int32), ids[r] - lo))
        s = np.where(rvalid & (rids >= lo) & (rids < hi))[0]
        vv_sel.append((root_flat_idx[s].astype(np.int32), rids[s] - lo, s))

    L_in, B_in = _window_schedule([x[1] for x in intra_sel], nwin_f, W_IN)
    L_g, B_g = _window_schedule([x[1] for x in glob_sel], nwin_c, W_G)
    L_xs, B_xs = _window_schedule([x[1] for x in xs_sel], nwin_xs, W_XS)
    L_vv, B_vv = _window_schedule([x[1] for x in vv_sel], nwin_vv, W_VV)

    Tkk = (SS + P - 1) // P
    Tfin = (FLAT + P - 1) // P

    h16_full = h_flat.astype(BF16NP)
    in_maps = []
    for c in range(NC):
        m = {}
        m["hflat"] = h_flat
        m["h16"] = h16_full
        m["hflatT"] = np.ascontiguousarray(
            h_flat[c * FLAT : (c + 1) * FLAT].T
        ).astype(BF16NP)

        s_, dl, e = intra_sel[c]
        a = _build_edge_arrays(s_, dl, ea_flat[e], L_in, B_in, W_IN)
        m["idx_in"], m["rel_in"] = a["idx"], a["rel"]
        m["ea_in"] = a["pay"].astype(BF16NP)

        s_, dl, e = glob_sel[c]
        a = _build_edge_arrays(s_, dl, edge_attr[e], L_g, B_g, W_G)
        m["idx_g"], m["rel_g"] = a["idx"], a["rel"]
        m["ea_g"] = a["pay"].astype(BF16NP)

        r, dl = xs_sel[c]
        a = _build_edge_arrays(r, dl, None, L_xs, B_xs, W_XS, recipw=recip[ids[r]])
        m["idx_xs"], m["rel_xs"], m["w_xs"] = a["idx"], a["rel"], a["w"]

        r, dl, s = vv_sel[c]
        a = _build_edge_arrays(r, dl, None, L_vv, B_vv, W_VV, recipw=rrecip[rids[s]])
        m["idx_vv"], m["rel_vv"], m["w_vv"] = a["idx"], a["rel"], a["w"]

        kk_idx = np.zeros(Tkk * P, np.int32)
        kk_idx[:SS] = root_flat_idx[c * SS : (c + 1) * SS]
        m["idx_kk"] = _pack_rows(kk_idx, Tkk)

        fin_T = np.zeros(Tfin * P, np.int32)
        fin_K = np.zeros(Tfin * P, np.int32)
        fin_V = np.zeros(Tfin * P, np.float32)
        sl = slice(c * FLAT, (c + 1) * FLAT)
        fin_T[:FLAT] = ids[sl]
        fin_K[:FLAT] = sub_batch[sl]
        fin_V[:FLAT] = valid[sl].astype(np.float32)
        m["idx_T"] = _pack_rows(fin_T, Tfin)
        m["idx_K"] = _pack_rows(fin_K, Tfin)
        m["val_f"] = _pack_rows(fin_V, Tfin)

        m["ident"] = np.eye(P, dtype=np.float32)
        m["iota8"] = np.tile(np.arange(MAXW, dtype=np.float32), (P, 8))
        mats = np.stack(
            [
                np.asarray(inputs["local_w1"], np.float32),
                np.asarray(inputs["local_w2"], np.float32),
                np.asarray(inputs["global_w1"], np.float32),
                np.asarray(inputs["global_w2"], np.float32),
                np.asarray(inputs["skip_w"], np.float32),
                np.asarray(inputs["vv_w"], np.float32),
                np.asarray(inputs["kk_w"], np.float32),
            ],
            axis=1,
        )  # [H, 7, H]
        m["mats"] = np.ascontiguousarray(mats.reshape(H, 7 * H))
        cb = (
            np.asarray(inputs["skip_b"], np.float32)
            + np.asarray(inputs["vv_b"], np.float32)
            + np.asarray(inputs["kk_b"], np.float32)
        )
        vecs = np.stack(
            [
                np.asarray(inputs["local_b1"], np.float32),
                np.asarray(inputs["global_b1"], np.float32),
                np.asarray(inputs["local_gamma"], np.float32),
                np.asarray(inputs["local_beta"], np.float32),
                np.asarray(inputs["global_gamma"], np.float32),
                np.asarray(inputs["global_beta"], np.float32),
                cb,
                np.asarray(inputs["local_b2"], np.float32),
                np.asarray(inputs["global_b2"], np.float32),
            ],
            axis=1,
        )  # [H, 9]
        m["vecs"] = np.ascontiguousarray(vecs)
        in_maps.append(m)

    sched = {
        "sizes": sizes,
        "FLAT": FLAT, "CAN": CAN, "SS": SS,
        "nwin_f": nwin_f, "nwin_c": nwin_c,
        "ops_in": _matmul_ops(L_in, B_in, nwin_f),
        "ops_g": _matmul_ops(L_g, B_g, nwin_c),
        "ops_xs": _matmul_ops(L_xs, B_xs, nwin_xs),
        "ops_vv": _matmul_ops(L_vv, B_vv, nwin_vv),
        "T_in": in_maps[0]["idx_in"].shape[1],
        "T_g": in_maps[0]["idx_g"].shape[1],
        "T_xs": in_maps[0]["idx_xs"].shape[1],
        "T_vv": in_maps[0]["idx_vv"].shape[1],
        "T_kk": Tkk, "T_fin": Tfin,
    }
    return in_maps, sched


# ---------------------------------------------------------------- graph build
def _win_width(w, win, total):
    return min(win, total - w * win)


def _group_list(total, win=P):
    """Groups of up to 512//win windows (<=512 node columns each)."""
    nwin = (total + win - 1) // win
    per = max(1, 512 // win)
    groups = []
    w = 0
    while w < nwin:
        ws = list(range(w, min(w + per, nwin)))
        width = sum(_win_width(x, win, total) for x in ws)
        groups.append((ws, ws[0] * win, width))
        w += per
    return groups


def _build(sched, in_maps):
    sizes = sched["sizes"]
    NF, NT, S = sizes["NF"], sizes["NT"], sizes["S"]
    FLAT, CAN, SS = sched["FLAT"], sched["CAN"], sched["SS"]
    T_in, T_g, T_xs, T_vv = sched["T_in"], sched["T_g"], sched["T_xs"], sched["T_vv"]
    T_kk, T_fin = sched["T_kk"], sched["T_fin"]

    nc = bass.Bass()

    def din(name, arr):
        if arr.dtype == np.float32:
            dt = F32
        elif arr.dtype == BF16NP:
            dt = BF16
        else:
            dt = I32
        return nc.declare_dram_parameter(name, list(arr.shape), dt,
                                         isOutput=False)

    m0 = in_maps[0]
    hflat = din("hflat", m0["hflat"])
    h16 = din("h16", m0["h16"])
    hflatT = din("hflatT", m0["hflatT"])
    idx_in, rel_in, ea_in = din("idx_in", m0["idx_in"]), din("rel_in", m0["rel_in"]), din("ea_in", m0["ea_in"])
    idx_g, rel_g, ea_g = din("idx_g", m0["idx_g"]), din("rel_g", m0["rel_g"]), din("ea_g", m0["ea_g"])
    idx_xs, rel_xs, w_xs = din("idx_xs", m0["idx_xs"]), din("rel_xs", m0["rel_xs"]), din("w_xs", m0["w_xs"])
    idx_vv, rel_vv, w_vv = din("idx_vv", m0["idx_vv"]), din("rel_vv", m0["rel_vv"]), din("w_vv", m0["w_vv"])
    idx_kk = din("idx_kk", m0["idx_kk"])
    idx_T, idx_K, val_f = din("idx_T", m0["idx_T"]), din("idx_K", m0["idx_K"]), din("val_f", m0["val_f"])
    mats, vecs = din("mats", m0["mats"]), din("vecs", m0["vecs"])
    ident_d, iota_d = din("ident", m0["ident"]), din("iota8", m0["iota8"])
    out_ext = nc.declare_dram_parameter("out", [FLAT, H], F32, isOutput=True)

    # internal DRAM scratch
    t1T = nc.dram_tensor("t1T", [P, FLAT], BF16)
    t2T = nc.dram_tensor("t2T", [P, CAN], F32)
    xsT_loc = nc.dram_tensor("xsT_loc", [P, CAN], BF16)
    xs_bounce = nc.dram_tensor("xs_bounce", [CAN, H], BF16)
    xs_full = nc.dram_tensor("xs_full", [NT, H], BF16, addr_space="Shared")
    vvT = nc.dram_tensor("vvT", [P, CAN], F32)
    kk_bounce = nc.dram_tensor("kk_bounce", [SS, H], BF16)
    kk_full = nc.dram_tensor("kk_full", [S, H], BF16, addr_space="Shared")
    T_bounce = nc.dram_tensor("T_bounce", [CAN, H], BF16)
    T_full = nc.dram_tensor("T_full", [NT, H], BF16, addr_space="Shared")
    kg_stage = nc.dram_tensor("kg_stage", [FLAT, H], BF16)
    st_in = nc.dram_tensor("st_in", [P, 4], F32)
    st_out = nc.dram_tensor("st_out", [P, 4], F32, addr_space="Shared")

    RG = [list(range(NC))]
    NB = 8  # tiles per gather batch
    PH = os.environ.get("K_PHASES", "ABCDEF")

    with tile.TileContext(nc) as tc:
        with (
            tc.tile_pool(name="const", bufs=1) as cp,
            tc.tile_pool(name="stats", bufs=1) as statp,
        ):
            ident_t = cp.tile([P, P], F32)
            nc.sync.dma_start(out=ident_t[:], in_=ident_d[:])
            identb = cp.tile([P, P], BF16)
            nc.vector.tensor_copy(out=identb[:], in_=ident_t[:])
            mats_b = cp.tile([P, 7 * H], BF16)
            iota8 = cp.tile([P, NB * MAXW], F32)
            nc.sync.dma_start(out=iota8[:], in_=iota_d[:])
            mats_t = cp.tile([P, 7 * H], F32)
            nc.sync.dma_start(out=mats_t[:], in_=mats[:])
            nc.vector.tensor_copy(out=mats_b[:], in_=mats_t[:])
            vecs_t = cp.tile([P, 9], F32)
            nc.sync.dma_start(out=vecs_t[:], in_=vecs[:])
            W1L, W2L = mats_b[:, 0*H:1*H], mats_b[:, 1*H:2*H]
            W1G, W2G = mats_b[:, 2*H:3*H], mats_b[:, 3*H:4*H]
            WSK, WVV, WKK = mats_b[:, 4*H:5*H], mats_b[:, 5*H:6*H], mats_b[:, 6*H:7*H]
            B1L, B1G = vecs_t[:, 0:1], vecs_t[:, 1:2]
            GAL, BEL = vecs_t[:, 2:3], vecs_t[:, 3:4]
            GAG, BEG = vecs_t[:, 4:5], vecs_t[:, 5:6]
            CB = vecs_t[:, 6:7]

            # strips for BN statistics (sum / sum of squares per group)
            ngrp_f = len(_group_list(FLAT))
            ngrp_c = len(_group_list(CAN))
            s1_strip = statp.tile([P, ngrp_f], F32)
            q1_strip = statp.tile([P, ngrp_f], F32)
            s2_strip = statp.tile([P, ngrp_c], F32)
            q2_strip = statp.tile([P, ngrp_c], F32)

            # ============================================================
            # generic scatter-conv phase
            # ============================================================
            def scatter_phase(
                prefix, Ttiles, idx_d, rel_d, table, ops, total_cols,
                ea_d=None, w_d=None, win=P, psum_bufs=4, tdt=F32,
            ):
                """Gathers rows, builds messages, one-hot scatter-matmuls into
                PSUM groups; calls group_done(g, psum_ap, width, base) when a
                4-window group is complete."""
                with (
                    tc.tile_pool(name=prefix + "g", bufs=12) as gp,
                    tc.tile_pool(name=prefix + "m", bufs=8) as mp,
                    tc.tile_pool(name=prefix + "o", bufs=8) as op_,
                    tc.tile_pool(name=prefix + "i", bufs=1) as ip,
                    tc.tile_pool(name=prefix + "ps", bufs=psum_bufs, space="PSUM") as pp,
                ):
                    idx_t = ip.tile([P, Ttiles], I32)
                    nc.sync.dma_start(out=idx_t[:], in_=idx_d[:])
                    rel_t = ip.tile([P, Ttiles], F32)
                    nc.sync.dma_start(out=rel_t[:], in_=rel_d[:])
                    if w_d is not None:
                        w_t = ip.tile([P, Ttiles], F32)
                        nc.sync.dma_start(out=w_t[:], in_=w_d[:])

                    nbatch = (Ttiles + NB - 1) // NB
                    msgs_tiles = [None] * Ttiles
                    state = {"next": 0}

                    def issue_batch(b):
                        t0 = b * NB
                        nt = min(NB, Ttiles - t0)
                        g = gp.tile([P, NB, H], BF16, tag="g")
                        for j in range(nt):
                            nc.gpsimd.indirect_dma_start(
                                out=g[:, j, :], out_offset=None, in_=table[:],
                                in_offset=bass.IndirectOffsetOnAxis(
                                    ap=idx_t[:, t0 + j : t0 + j + 1], axis=0
                                ),
                            )
                        if ea_d is not None:
                            ea_t = mp.tile([P, NB, H], BF16, tag="ea")
                            nc.sync.dma_start(
                                out=ea_t[:, :nt, :], in_=ea_d[:, t0 : t0 + nt, :]
                            )
                            ms = mp.tile([P, NB, H], BF16, tag="ms")
                            nc.vector.tensor_tensor(
                                out=ms[:, :nt, :], in0=g[:, :nt, :],
                                in1=ea_t[:, :nt, :], op=mybir.AluOpType.add,
                            )
                            nc.scalar.activation(
                                out=ms[:, :nt, :], in_=ms[:, :nt, :],
                                func=mybir.ActivationFunctionType.Relu,
                            )
                        elif w_d is not None:
                            ms = mp.tile([P, NB, H], BF16, tag="ms")
                            nc.vector.tensor_tensor(
                                out=ms[:, :nt, :], in0=g[:, :nt, :],
                                in1=w_t[:, t0 : t0 + nt, None].to_broadcast(
                                    [P, nt, H]
                                ),
                                op=mybir.AluOpType.mult,
                            )
                        else:
                            ms = mp.tile([P, NB, H], BF16, tag="ms")
                            nc.vector.tensor_copy(out=ms[:, :nt, :],
                                                  in_=g[:, :nt, :])
                        oh = op_.tile([P, NB, win], BF16, tag="oh")
                        nc.vector.tensor_tensor(
                            out=oh[:, :nt, :],
                            in0=rel_t[:, t0 : t0 + nt, None].to_broadcast(
                                [P, nt, win]
                            ),
                            in1=iota8[:].rearrange("p (n q) -> p n q", n=NB)[
                                :, :nt, :win
                            ],
                            op=mybir.AluOpType.is_equal,
                        )
                        for j in range(nt):
                            msgs_tiles[t0 + j] = (ms, j, oh)

                    def ensure_tiles(tmax):
                        while state["next"] * NB <= tmax:
                            issue_batch(state["next"])
                            state["next"] += 1

                    groups = _group_list(total_cols, win)
                    opi = 0
                    for gi, (ws, base, width) in enumerate(groups):
                        ps = pp.tile([P, 512], F32, tag="agg")
                        for w in ws:
                            wslot = (w - ws[0]) * win
                            wn = _win_width(w, win, total_cols)
                            while opi < len(ops) and ops[opi][3] == w:
                                (t, a, b, _w, st, sp) = ops[opi]
                                ensure_tiles(t)
                                ms, j, oh = msgs_tiles[t]
                                nc.tensor.matmul(
                                    out=ps[:, wslot : wslot + wn],
                                    lhsT=ms[a:b, j, :],
                                    rhs=oh[a:b, j, :wn],
                                    start=st, stop=sp,
                                )
                                opi += 1
                        yield gi, ps, base, width

            # ============================================================
            # Phase A: x_sum build
            # ============================================================
            if "A" in PH:
              with (
                tc.tile_pool(name="xa", bufs=3) as xa,
                tc.tile_pool(name="xps", bufs=2, space="PSUM") as xps,
              ):
                for gi, ps, base, width in scatter_phase(
                    "xs", T_xs, idx_xs, rel_xs, h16, sched["ops_xs"], CAN,
                    w_d=w_xs, win=W_XS,
                ):
                    xs_sbb = xa.tile([P, 512], BF16, tag="xs_sbb")
                    nc.vector.tensor_copy(out=xs_sbb[:, :width],
                                          in_=ps[:, :width])
                    nc.scalar.dma_start(
                        out=xsT_loc[:, base : base + width],
                        in_=xs_sbb[:, :width]
                    )
                    # transpose to node-major rows for the AllGather table
                    nsub = (width + P - 1) // P
                    tr = xa.tile([P, nsub, P], BF16, tag="xs_tr")
                    for s in range(nsub):
                        wn = min(P, width - s * P)
                        pst = xps.tile([P, P], F32, tag="xs_t")
                        nc.tensor.matmul(
                            out=pst[:wn, :], lhsT=xs_sbb[:, s * P : s * P + wn],
                            rhs=identb[:], start=True, stop=True,
                        )
                        nc.scalar.activation(
                            out=tr[:wn, s, :], in_=pst[:wn, :],
                            func=mybir.ActivationFunctionType.Copy,
                        )
                        nc.scalar.dma_start(
                            out=xs_bounce[base + s * P : base + s * P + wn, :],
                            in_=tr[:wn, s, :],
                        )
            if "D" in PH:
              with (
                tc.tile_pool(name="dv", bufs=6) as dv,
                tc.tile_pool(name="dps", bufs=2, space="PSUM") as dps,
              ):
                for gi, ps, base, width in scatter_phase(
                    "vv", T_vv, idx_vv, rel_vv, h16, sched["ops_vv"], CAN,
                    w_d=w_vv, win=W_VV, psum_bufs=2,
                ):
                    xv = dv.tile([P, 512], BF16, tag="xv")
                    nc.scalar.activation(
                        out=xv[:, :width], in_=ps[:, :width],
                        func=mybir.ActivationFunctionType.Copy,
                    )
                    psv = dps.tile([P, 512], F32, tag="psv")
                    nc.tensor.matmul(out=psv[:, :width], lhsT=WVV,
                                     rhs=xv[:, :width], start=True, stop=True)
                    vvc = dv.tile([P, 512], F32, tag="vvc")
                    nc.scalar.activation(
                        out=vvc[:, :width], in_=psv[:, :width],
                        func=mybir.ActivationFunctionType.Copy,
                    )
                    nc.scalar.dma_start(
                        out=vvT[:, base : base + width], in_=vvc[:, :width]
                    )

                # A's bounce stores have drained by now; the AllGather issue
                # on Pool no longer stalls
                nc.gpsimd.collective_compute(
                    "AllGather", mybir.AluOpType.bypass,
                    ins=[xs_bounce[:].opt()],
                    outs=[xs_full[:].opt()], replica_groups=RG,
                )

                # kk: gather root rows (node-major), transpose, matmul, store
                idxk_t = dv.tile([P, T_kk], I32, tag="idxk")
                nc.sync.dma_start(out=idxk_t[:], in_=idx_kk[:])
                for b in range((T_kk + NB - 1) // NB):
                    t0 = b * NB
                    nt = min(NB, T_kk - t0)
                    gk = dv.tile([P, NB, H], BF16, tag="gk")
                    for j in range(nt):
                        nc.gpsimd.indirect_dma_start(
                            out=gk[:, j, :], out_offset=None, in_=h16[:],
                            in_offset=bass.IndirectOffsetOnAxis(
                                ap=idxk_t[:, t0 + j : t0 + j + 1], axis=0
                            ),
                        )
                    for j in range(nt):
                        t = t0 + j
                        pstr = dps.tile([P, P], F32, tag="pstr")
                        nc.tensor.matmul(out=pstr[:], lhsT=gk[:, j, :],
                                         rhs=identb[:], start=True, stop=True)
                        gkT = dv.tile([P, P], BF16, tag="gkT")
                        nc.scalar.activation(
                            out=gkT[:], in_=pstr[:],
                            func=mybir.ActivationFunctionType.Copy,
                        )
                        psk = dps.tile([P, P], F32, tag="psk")
                        nc.tensor.matmul(out=psk[:], lhsT=gkT[:], rhs=WKK,
                                         start=True, stop=True)
                        kkr = dv.tile([P, P], BF16, tag="kkr")
                        nc.scalar.activation(
                            out=kkr[:], in_=psk[:],
                            func=mybir.ActivationFunctionType.Copy,
                        )
                        nrow = min(P, SS - t * P)
                        nc.scalar.dma_start(
                            out=kk_bounce[t * P : t * P + nrow, :],
                            in_=kkr[:nrow, :],
                        )
              nc.gpsimd.collective_compute(
                "AllGather", mybir.AluOpType.bypass, ins=[kk_bounce[:].opt()],
                outs=[kk_full[:].opt()], replica_groups=RG,
              )

            # ============================================================
            # Phase B: intra conv -> t1T + stats
            # ============================================================
            def mlp_phase(gen, W1, W2, B1, xT_src, tT_dst, strip_s, strip_q,
                          mlp_pool, mlp_psum, odt=F32):
                for gi, ps, base, width in gen:
                    xc = mlp_pool.tile([P, 512], BF16, tag="xc")
                    nc.sync.dma_start(
                        out=xc[:, :width], in_=xT_src[:, base : base + width]
                    )
                    hT = mlp_pool.tile([P, 512], BF16, tag="hT")
                    nc.vector.tensor_tensor(
                        out=hT[:, :width], in0=ps[:, :width], in1=xc[:, :width],
                        op=mybir.AluOpType.add,
                    )
                    psy = mlp_psum.tile([P, 512], F32, tag="psy")
                    nc.tensor.matmul(out=psy[:, :width], lhsT=W1,
                                     rhs=hT[:, :width], start=True, stop=True)
                    y1 = mlp_pool.tile([P, 512], BF16, tag="y1")
                    nc.scalar.activation(
                        out=y1[:, :width], in_=psy[:, :width],
                        func=mybir.ActivationFunctionType.Relu, bias=B1,
                    )
                    pst = mlp_psum.tile([P, 512], F32, tag="pst")
                    nc.tensor.matmul(out=pst[:, :width], lhsT=W2,
                                     rhs=y1[:, :width], start=True, stop=True)
                    tt = mlp_pool.tile([P, 512], odt, tag="tt")
                    nc.scalar.activation(
                        out=tt[:, :width], in_=pst[:, :width],
                        func=mybir.ActivationFunctionType.Copy,
                        accum_out=strip_s[:, gi : gi + 1],
                    )
                    sq = mlp_pool.tile([P, 512], odt, tag="sq")
                    nc.scalar.activation(
                        out=sq[:, :width], in_=tt[:, :width],
                        func=mybir.ActivationFunctionType.Square,
                        accum_out=strip_q[:, gi : gi + 1],
                    )
                    nc.scalar.dma_start(
                        out=tT_dst[:, base : base + width], in_=tt[:, :width]
                    )

            if "B" in PH:
              with (
                tc.tile_pool(name="bm", bufs=3) as bm,
                tc.tile_pool(name="bps", bufs=2, space="PSUM") as bps,
              ):
                mlp_phase(
                    scatter_phase("in", T_in, idx_in, rel_in, h16,
                                  sched["ops_in"], FLAT, ea_d=ea_in, win=W_IN),
                    W1L, W2L, B1L, hflatT, t1T, s1_strip, q1_strip, bm, bps,
                    odt=BF16,
                )

            # ============================================================
            # Phase C: global conv -> t2T + stats
            # ============================================================
            if "C" in PH:
              with (
                tc.tile_pool(name="cm", bufs=3) as cm,
                tc.tile_pool(name="cps", bufs=2, space="PSUM") as cps,
              ):
                mlp_phase(
                    scatter_phase("gl", T_g, idx_g, rel_g, xs_full,
                                  sched["ops_g"], CAN, ea_d=ea_g, win=W_G,
                                  tdt=BF16),
                    W1G, W2G, B1G, xsT_loc, t2T, s2_strip, q2_strip, cm, cps,
                )

            # ============================================================
            # Phase D: vv scatter-mean + matmul; kk rows
            # ============================================================
            # ============================================================
            # Phase E: BN stats AllReduce, T build, AG T
            # ============================================================
            if "E" in PH:
              with (
                tc.tile_pool(name="ev", bufs=2) as ev,
                tc.tile_pool(name="ekg", bufs=4) as ekg,
                tc.tile_pool(name="efi", bufs=1) as efi,
                tc.tile_pool(name="eps", bufs=2, space="PSUM") as eps,
              ):
                idxT_t = efi.tile([P, T_fin], I32)
                nc.sync.dma_start(out=idxT_t[:], in_=idx_T[:])
                idxK_t = efi.tile([P, T_fin], I32)
                nc.sync.dma_start(out=idxK_t[:], in_=idx_K[:])
                valf_t = efi.tile([P, T_fin], F32)
                nc.sync.dma_start(out=valf_t[:], in_=val_f[:])

                def kg_stage_tiles(tlo, thi):
                    """Gather kk rows (bf16) into DRAM staging, batched."""
                    for b0 in range(tlo, thi, NB):
                        nt = min(NB, thi - b0)
                        kg = ekg.tile([P, NB, H], BF16, tag="kg")
                        for j in range(nt):
                            nc.gpsimd.indirect_dma_start(
                                out=kg[:, j, :], out_offset=None,
                                in_=kk_full[:],
                                in_offset=bass.IndirectOffsetOnAxis(
                                    ap=idxK_t[:, b0 + j : b0 + j + 1], axis=0),
                            )
                        r0 = b0 * P
                        rows = min(nt * P, FLAT - r0)
                        if rows == nt * P:
                            nc.scalar.dma_start(
                                out=kg_stage[r0 : r0 + rows, :].rearrange(
                                    "(j p) h -> p j h", p=P),
                                in_=kg[:, :nt, :],
                            )
                        else:
                            for j in range(nt):
                                nr = min(P, FLAT - r0 - j * P)
                                if nr <= 0:
                                    break
                                nc.scalar.dma_start(
                                    out=kg_stage[
                                        r0 + j * P : r0 + j * P + nr, :
                                    ],
                                    in_=kg[:nr, j, :],
                                )

                st = ev.tile([P, 4], F32, tag="st")
                nc.vector.tensor_reduce(out=st[:, 0:1], in_=s1_strip[:],
                                        axis=mybir.AxisListType.X,
                                        op=mybir.AluOpType.add)
                nc.vector.tensor_reduce(out=st[:, 1:2], in_=q1_strip[:],
                                        axis=mybir.AxisListType.X,
                                        op=mybir.AluOpType.add)
                nc.vector.tensor_reduce(out=st[:, 2:3], in_=s2_strip[:],
                                        axis=mybir.AxisListType.X,
                                        op=mybir.AluOpType.add)
                nc.vector.tensor_reduce(out=st[:, 3:4], in_=q2_strip[:],
                                        axis=mybir.AxisListType.X,
                                        op=mybir.AluOpType.add)
                nc.sync.dma_start(out=st_in[:], in_=st[:])
                nc.gpsimd.collective_compute(
                    "AllReduce", mybir.AluOpType.add, ins=[st_in[:].opt()],
                    outs=[st_out[:].opt()], replica_groups=RG,
                )
                # keep Pool busy during the AR + BN-math + T-build chain
                KG1 = min(T_fin, (T_fin // 2 + NB - 1) // NB * NB)
                kg_stage_tiles(0, KG1)
                sg = ev.tile([P, 4], F32, tag="sg")
                nc.sync.dma_start(out=sg[:], in_=st_out[:])
                # stats -> scale/shift vectors  [P,1] each
                wk = ev.tile([P, 10], F32, tag="wk")
                EPS = 1e-5
                # mu1 = sg0/NF ; var1 = sg1/NF - mu1^2
                nc.vector.tensor_scalar(out=wk[:, 0:1], in0=sg[:, 0:1],
                                        scalar1=1.0 / NF, scalar2=None,
                                        op0=mybir.AluOpType.mult)
                nc.vector.tensor_scalar(out=wk[:, 1:2], in0=sg[:, 1:2],
                                        scalar1=1.0 / NF, scalar2=None,
                                        op0=mybir.AluOpType.mult)
                nc.vector.tensor_tensor(out=wk[:, 2:3], in0=wk[:, 0:1],
                                        in1=wk[:, 0:1], op=mybir.AluOpType.mult)
                nc.vector.tensor_tensor(out=wk[:, 2:3], in0=wk[:, 1:2],
                                        in1=wk[:, 2:3],
                                        op=mybir.AluOpType.subtract)
                # rs1 = 1/sqrt(var1+eps)
                nc.vector.tensor_scalar(out=wk[:, 2:3], in0=wk[:, 2:3],
                                        scalar1=EPS, scalar2=None,
                                        op0=mybir.AluOpType.add)
                nc.scalar.activation(out=wk[:, 3:4], in_=wk[:, 2:3],
                                     func=mybir.ActivationFunctionType.Sqrt)
                nc.vector.reciprocal(out=wk[:, 3:4], in_=wk[:, 3:4])
                # s1 = gamma1*rs1 ; c1 = beta1 - mu1*s1
                s1v = ev.tile([P, 1], F32, tag="s1v")
                nc.vector.tensor_tensor(out=s1v[:], in0=GAL, in1=wk[:, 3:4],
                                        op=mybir.AluOpType.mult)
                c1v = ev.tile([P, 1], F32, tag="c1v")
                nc.vector.tensor_tensor(out=c1v[:], in0=wk[:, 0:1], in1=s1v[:],
                                        op=mybir.AluOpType.mult)
                nc.vector.tensor_tensor(out=c1v[:], in0=BEL, in1=c1v[:],
                                        op=mybir.AluOpType.subtract)
                # same for stats2
                nc.vector.tensor_scalar(out=wk[:, 4:5], in0=sg[:, 2:3],
                                        scalar1=1.0 / NT, scalar2=None,
                                        op0=mybir.AluOpType.mult)
                nc.vector.tensor_scalar(out=wk[:, 5:6], in0=sg[:, 3:4],
                                        scalar1=1.0 / NT, scalar2=None,
                                        op0=mybir.AluOpType.mult)
                nc.vector.tensor_tensor(out=wk[:, 6:7], in0=wk[:, 4:5],
                                        in1=wk[:, 4:5], op=mybir.AluOpType.mult)
                nc.vector.tensor_tensor(out=wk[:, 6:7], in0=wk[:, 5:6],
                                        in1=wk[:, 6:7],
                                        op=mybir.AluOpType.subtract)
                nc.vector.tensor_scalar(out=wk[:, 6:7], in0=wk[:, 6:7],
                                        scalar1=EPS, scalar2=None,
                                        op0=mybir.AluOpType.add)
                nc.scalar.activation(out=wk[:, 7:8], in_=wk[:, 6:7],
                                     func=mybir.ActivationFunctionType.Sqrt)
                nc.vector.reciprocal(out=wk[:, 7:8], in_=wk[:, 7:8])
                s2v = ev.tile([P, 1], F32, tag="s2v")
                nc.vector.tensor_tensor(out=s2v[:], in0=GAG, in1=wk[:, 7:8],
                                        op=mybir.AluOpType.mult)
                c2v = ev.tile([P, 1], F32, tag="c2v")
                nc.vector.tensor_tensor(out=c2v[:], in0=wk[:, 4:5], in1=s2v[:],
                                        op=mybir.AluOpType.mult)
                nc.vector.tensor_tensor(out=c2v[:], in0=BEG, in1=c2v[:],
                                        op=mybir.AluOpType.subtract)
                # Tconst = c2 + c1 + (skip_b+vv_b+kk_b)
                tcv = ev.tile([P, 1], F32, tag="tcv")
                nc.vector.tensor_tensor(out=tcv[:], in0=c2v[:], in1=c1v[:],
                                        op=mybir.AluOpType.add)
                nc.vector.tensor_tensor(out=tcv[:], in0=tcv[:], in1=CB,
                                        op=mybir.AluOpType.add)

                # T = s2*t2 + Tconst + vv  (feature-major), transpose, store
                for (ws, base, width) in _group_list(CAN):
                    t2c = ev.tile([P, 512], F32, tag="t2c")
                    nc.sync.dma_start(out=t2c[:, :width],
                                      in_=t2T[:, base : base + width])
                    vvc2 = ev.tile([P, 512], F32, tag="vvc2")
                    nc.sync.dma_start(out=vvc2[:, :width],
                                      in_=vvT[:, base : base + width])
                    nc.vector.tensor_scalar(
                        out=t2c[:, :width], in0=t2c[:, :width],
                        scalar1=s2v[:], scalar2=tcv[:],
                        op0=mybir.AluOpType.mult, op1=mybir.AluOpType.add,
                    )
                    nc.vector.tensor_tensor(out=t2c[:, :width],
                                            in0=t2c[:, :width],
                                            in1=vvc2[:, :width],
                                            op=mybir.AluOpType.add)
                    t2b = ev.tile([P, 512], BF16, tag="t2b")
                    nc.vector.tensor_copy(out=t2b[:, :width], in_=t2c[:, :width])
                    nsub = (width + P - 1) // P
                    trT = ev.tile([P, 4, P], BF16, tag="trT")
                    for s in range(nsub):
                        wn = min(P, width - s * P)
                        pst2 = eps.tile([P, P], F32, tag="pst2")
                        nc.tensor.matmul(out=pst2[:wn, :],
                                         lhsT=t2b[:, s * P : s * P + wn],
                                         rhs=identb[:], start=True, stop=True)
                        nc.scalar.activation(
                            out=trT[:wn, s, :], in_=pst2[:wn, :],
                            func=mybir.ActivationFunctionType.Copy,
                        )
                        nc.scalar.dma_start(
                            out=T_bounce[base + s * P : base + s * P + wn, :],
                            in_=trT[:wn, s, :],
                        )
                nc.gpsimd.collective_compute(
                    "AllGather", mybir.AluOpType.bypass, ins=[T_bounce[:].opt()],
                    outs=[T_full[:].opt()], replica_groups=RG,
                )
                # rest of the kg staging runs while the T AllGather is in
                # flight
                kg_stage_tiles(KG1, T_fin)

                # ========================================================
                # Phase F: final fuse, NBF tiles per gather batch
                # ========================================================
                if "F" in PH:
                  NBF = 8
                  with (
                    tc.tile_pool(name="fv", bufs=3) as fv,
                    tc.tile_pool(name="fo", bufs=4) as fo,
                    tc.tile_pool(name="fps", bufs=4, space="PSUM") as fps,
                  ):
                    nsg = (T_fin + NBF - 1) // NBF

                    def sg_tiles(sg):
                        t0 = sg * NBF
                        return t0, min(NBF, T_fin - t0)

                    for sg in range(nsg):
                        t0, nt = sg_tiles(sg)
                        tg = fv.tile([P, NBF, H], BF16, tag="tg")
                        for j in range(nt):
                            nc.gpsimd.indirect_dma_start(
                                out=tg[:, j, :], out_offset=None, in_=T_full[:],
                                in_offset=bass.IndirectOffsetOnAxis(
                                    ap=idxT_t[:, t0 + j : t0 + j + 1], axis=0),
                            )
                        kg = fv.tile([P, NBF, H], BF16, tag="kg")
                        r0k = t0 * P
                        rowsk = min(nt * P, FLAT - r0k)
                        if rowsk == nt * P:
                            nc.sync.dma_start(
                                out=kg[:, :nt, :],
                                in_=kg_stage[r0k : r0k + rowsk, :].rearrange(
                                    "(j p) h -> p j h", p=P),
                            )
                        else:
                            for j in range(nt):
                                nr = min(P, FLAT - r0k - j * P)
                                if nr <= 0:
                                    break
                                nc.sync.dma_start(
                                    out=kg[:nr, j, :],
                                    in_=kg_stage[
                                        r0k + j * P : r0k + j * P + nr, :
                                    ],
                                )
                        ncol = min(NBF * P, FLAT - t0 * P)
                        xc2 = fv.tile([P, NBF * P], BF16, tag="xc2")
                        nc.sync.dma_start(
                            out=xc2[:, :ncol],
                            in_=hflatT[:, t0 * P : t0 * P + ncol])
                        t1c = fv.tile([P, NBF * P], BF16, tag="t1c")
                        nc.sync.dma_start(
                            out=t1c[:, :ncol],
                            in_=t1T[:, t0 * P : t0 * P + ncol])
                        y1c = fv.tile([P, NBF * P], BF16, tag="y1c")
                        nc.scalar.activation(
                            out=y1c[:, :ncol], in_=t1c[:, :ncol],
                            func=mybir.ActivationFunctionType.Copy,
                            scale=s1v[:],
                        )
                        for q in range(0, nt, 4):
                            qt = min(4, nt - q)
                            psf = fps.tile([P, 512], F32, tag="psf")
                            for j in range(q, q + qt):
                                c = (j - q) * P
                                nc.tensor.matmul(
                                    out=psf[:, c : c + P],
                                    lhsT=xc2[:, j * P : (j + 1) * P], rhs=WSK,
                                    start=True, stop=False)
                                nc.tensor.matmul(
                                    out=psf[:, c : c + P],
                                    lhsT=y1c[:, j * P : (j + 1) * P],
                                    rhs=identb[:], start=False, stop=True)
                            u = fo.tile([P, 4, H], F32, tag="u")
                            nc.vector.tensor_tensor(
                                out=u[:, :qt, :], in0=tg[:, q : q + qt, :],
                                in1=kg[:, q : q + qt, :],
                                op=mybir.AluOpType.add)
                            nc.vector.tensor_tensor(
                                out=u[:, :qt, :], in0=u[:, :qt, :],
                                in1=psf[:, : qt * P].rearrange(
                                    "p (j h) -> p j h", j=qt),
                                op=mybir.AluOpType.add)
                            nc.vector.tensor_tensor(
                                out=u[:, :qt, :], in0=u[:, :qt, :],
                                in1=valf_t[
                                    :, t0 + q : t0 + q + qt, None
                                ].to_broadcast([P, qt, H]),
                                op=mybir.AluOpType.mult)
                            ob = fo.tile([P, 4, H], F32, tag="ob")
                            nc.scalar.activation(
                                out=ob[:, :qt, :], in_=u[:, :qt, :],
                                func=mybir.ActivationFunctionType.Relu,
                            )
                            r0 = (t0 + q) * P
                            rows = min(qt * P, FLAT - r0)
                            if rows == qt * P:
                                nc.scalar.dma_start(
                                    out=out_ext[r0 : r0 + rows, :].rearrange(
                                        "(j p) h -> p j h", p=P),
                                    in_=ob[:, :qt, :],
                                )
                            else:
                                for j in range(qt):
                                    nr = min(P, FLAT - r0 - j * P)
                                    if nr <= 0:
                                        break
                                    nc.scalar.dma_start(
                                        out=out_ext[
                                            r0 + j * P : r0 + j * P + nr, :
                                        ],
                                        in_=ob[:nr, j, :],
                                    )

    _split_multi_waits(nc)
    return nc


# --------------------------------------------------------------------- driver
_DEFAULT_SIZES = dict(NF=320000, NT=100000, S=20000, K=16, EI=1280000, EG=800000)


LAST = {"exec_time_ns": None}


def kernel(sizes=None, **inputs):
    sizes = dict(_DEFAULT_SIZES if sizes is None else sizes)
    in_maps, sched = _prep(inputs, sizes)
    nc = _build(sched, in_maps)
    trace = bool(os.environ.get("K_TRACE"))
    kw = {}
    if trace and os.environ.get("K_TRACEDIR"):
        kw["tmpdir"] = os.environ["K_TRACEDIR"]
    res = run_bass_kernel_spmd(nc, in_maps, core_ids=list(range(NC)), trace=trace,
                               **kw)
    LAST["exec_time_ns"] = res.exec_time_ns
    LAST["res"] = res
    outs = [np.asarray(r["out"]) for r in res.results]
    return np.concatenate(outs, axis=0).astype(np.float32)

